# revision 17
# baseline (speedup 1.0000x reference)
"""Trainium2 Bass kernel for a bag-of-words model (EmbeddingBag mean ->
Linear -> BatchNorm(train stats) -> ReLU -> Linear).

Strategy (8 NeuronCores, SPMD, v2):
  - Host precomputes embW = emb @ W1 (pooling and W1 commute), quantizes it
    to fp8 e3m4 (x64 scale) -> gather traffic halves vs bf16 and the W1
    matmul disappears from the device entirely.  b1 cancels in BN's mean
    subtraction, so it is dropped.
  - Data-parallel over batch: 1024 examples -> 128 per core, balanced by
    token count.  Host buckets tokens by 32768-row vocab ranges (int16
    gather indices) and builds per-slot segment ids.
  - Device: dma_gather pulls the embW rows for each token into SBUF;
    pooling runs on TensorE as pooled[seg, H] += M^T @ G with the 0/1
    mask M built on DVE via is_equal(iota, seg) in e3m4.
  - Tail: pooled/(len*S) on DVE, PE transpose to feature-major PSUM, BN
    batch stats via DVE reduces, a one-shot 8-way stats exchange with
    remote_dma_broadcast (4KB to each peer; XOR-slot permutation is
    sum-invariant), fused BN+ReLU on ACT straight out of PSUM, final
    5-wide matmul, +b2, DMA out.
"""

import numpy as np
import ml_dtypes

B, L, V, H = 1024, 200, 100000, 512
NCORES = 8
PCORE = B // NCORES  # 128 examples per core
BUCKET = 32768  # dma_gather int16 index range per bucket
NBUCKETS = (V + BUCKET - 1) // BUCKET  # 4
BN_EPS = 1e-5
CHUNK_TILES = 8  # token-tiles per dma_gather call
SEG_PAD = 200.0  # segment id for padding slots (never matches 0..127)
FP8_SCALE = 64.0  # embW quantization scale for e3m4
GATHER_QUEUES = 3  # swdge queues 0..2 for gathers; queue 3 is the AR queue
AR_QUEUE = 3

_CACHE = {}

# AR_MODE: "remote" = one-shot remote_dma stats exchange (production);
# "nowait" = fire remote sends but don't wait (debug); "none" = no remote
# sends at all, per-core stats only (debug).
import os as _os

AR_MODE = _os.environ.get("KERNEL_AR_MODE", "remote")


# ----------------------------------------------------------------- host prep
def _assign_cores(lengths):
    """Balanced assignment: 128 examples per core, ~equal total tokens."""
    order = np.argsort(-lengths, kind="stable")
    loads = [0] * NCORES
    counts = [0] * NCORES
    cores = [[] for _ in range(NCORES)]
    for ex in order:
        c = min(
            (c for c in range(NCORES) if counts[c] < PCORE),
            key=lambda c: loads[c],
        )
        cores[c].append(int(ex))
        loads[c] += int(lengths[ex])
        counts[c] += 1
    return cores


def _prep(tokens, lengths):
    """Build per-core gather indices / segment ids / call plan."""
    cores = _assign_cores(lengths)

    # per (core, bucket): list of (idx16, seg)
    percb = [[[] for _ in range(NBUCKETS)] for _ in range(NCORES)]
    for c in range(NCORES):
        for slot, ex in enumerate(cores[c]):
            n = int(lengths[ex])
            toks = tokens[ex, :n]
            bs = toks >> 15
            rs = toks & 0x7FFF
            for b, r in zip(bs, rs):
                percb[c][b].append((int(r), slot))

    # shared padded bucket sizes (tiles), so all cores run the same program
    bsz = []
    for b in range(NBUCKETS):
        mx = max(len(percb[c][b]) for c in range(NCORES))
        bsz.append(-(-mx // 128) * 128)
    n_slots = sum(bsz)
    n_tiles = n_slots // 128

    # gather call plan: (bucket_base_row, tile_offset, n_tiles_call)
    calls = []
    t0 = 0
    for b in range(NBUCKETS):
        bt = bsz[b] // 128
        done = 0
        while done < bt:
            nt = min(CHUNK_TILES, bt - done)
            calls.append((b * BUCKET, t0 + done, nt))
            done += nt
        t0 += bt

    idx16 = np.zeros((NCORES, 128, n_slots // 16), dtype=np.int16)
    seg = np.zeros((NCORES, 128, n_tiles), dtype=np.float32)
    leninv = np.zeros((NCORES, 128, 1), dtype=np.float32)
    for c in range(NCORES):
        flat_idx = np.zeros(n_slots, dtype=np.int16)
        flat_seg = np.full(n_slots, SEG_PAD, dtype=np.float32)
        off = 0
        for b in range(NBUCKETS):
            lst = percb[c][b]
            if lst:
                arr = np.asarray(lst, dtype=np.int64)
                flat_idx[off : off + len(lst)] = arr[:, 0].astype(np.int16)
                flat_seg[off : off + len(lst)] = arr[:, 1]
            off += bsz[b]
        # wrap: slot k -> [k % 16, k // 16], replicated to 128 partitions
        w = flat_idx.reshape(n_slots // 16, 16).T  # [16, n_slots//16]
        idx16[c] = np.tile(w, (8, 1))
        seg[c] = flat_seg.reshape(n_tiles, 128).T  # [p, t]
        leninv[c, :, 0] = 1.0 / (
            lengths[np.asarray(cores[c])].astype(np.float32) * FP8_SCALE
        )

    return cores, calls, n_tiles, idx16, seg, leninv


# -------------------------------------------------------------- device build
def _build(n_tiles, calls):
    import concourse.bacc as bacc
    import concourse.tile as tile
    import concourse.mybir as mybir

    f32 = mybir.dt.float32
    f8 = mybir.dt.float8e3

    nc = bacc.Bacc(
        "TRN2",
        target_bir_lowering=False,
        debug=False,
        enable_asserts=False,
        num_devices=NCORES,
        dynamic_dma_scratch_size=32768,
        num_swdge_queues=4,
    )

    emb_d = nc.dram_tensor("embWq", [V, H], f8, kind="ExternalInput")
    idx_d = nc.dram_tensor(
        "idx16", [128, n_tiles * 8], mybir.dt.int16, kind="ExternalInput"
    )
    seg_d = nc.dram_tensor("seg", [128, n_tiles], f32, kind="ExternalInput")
    iota_d = nc.dram_tensor("iota", [128, 128], f32, kind="ExternalInput")
    cst_d = nc.dram_tensor("cst", [128, 14], f32, kind="ExternalInput")
    w2_d = nc.dram_tensor("W2p", [128, 20], f32, kind="ExternalInput")
    ident_d = nc.dram_tensor("ident", [128, 128], f32, kind="ExternalInput")
    out_d = nc.dram_tensor("out", [128, 5], f32, kind="ExternalOutput")


    with tile.TileContext(nc) as tc:
        with (
            tc.tile_pool(name="const", bufs=1) as cpool,
            tc.tile_pool(name="gbuf", bufs=4) as gpool,
            tc.tile_pool(name="work", bufs=1) as wpool,
            tc.tile_pool(name="ppool", bufs=1, space="PSUM") as ppool,
            tc.tile_pool(name="tpsum", bufs=1, space="PSUM") as tppool,
            tc.tile_pool(name="opsum", bufs=1, space="PSUM") as opool,
            tc.tile_pool(name="dram", bufs=1, space="DRAM") as dpool,
        ):
            # ---- constant / input loads (all small)
            idx_sb = cpool.tile([128, n_tiles * 8], mybir.dt.int16, tag="idx")
            seg_sb = cpool.tile([128, n_tiles], f32, tag="seg")
            iota_sb = cpool.tile([128, 128], f32, tag="iota")
            cst_sb = cpool.tile([128, 14], f32, tag="cst")
            ident_sb = cpool.tile([128, 128], f32, tag="ident")
            w2_sb = cpool.tile([128, 4, 5], f32, tag="w2")
            msk_sb = cpool.tile([128, n_tiles * 128], f8, tag="msk")
            stats_sb = cpool.tile([128, 8], f32, tag="stats")

            nc.sync.dma_start(idx_sb[:], idx_d[:, :])
            nc.sync.dma_start(seg_sb[:], seg_d[:, :])
            nc.sync.dma_start(iota_sb[:], iota_d[:, :])
            nc.sync.dma_start(cst_sb[:], cst_d[:, :])
            nc.sync.dma_start(ident_sb[:], ident_d[:, :])
            nc.sync.dma_start(w2_sb[:], w2_d[:, :])
            leninv_sb = cst_sb[:, 0:1]
            gamma_sb = cst_sb[:, 1:5]
            beta_sb = cst_sb[:, 5:9]
            b2_sb = cst_sb[:, 9:14]

            # ---- warm-up AllReduce: boot ncfw early so the real one is fast
            warm_in = dpool.tile([128, 1], f32, tag="warm_in")
            warm_out = dpool.tile([128, 1], f32, tag="warm_out")
            nc.sync.dma_start(warm_in[:], cst_sb[:, 0:1])
            nc.gpsimd.collective_compute(
                "AllReduce",
                mybir.AluOpType.add,
                replica_groups=[list(range(NCORES))],
                ins=[warm_in.opt()],
                outs=[warm_out.opt()],
            )

            # ---- masks: msk[p, t*128+s] = (seg[p, t] == s), in e3m4
            for base, t0, nt in calls:
                mv = msk_sb[:, t0 * 128 : (t0 + nt) * 128].rearrange(
                    "p (t s) -> p t s", s=128
                )
                nc.vector.tensor_tensor(
                    out=mv,
                    in0=iota_sb[:].unsqueeze(1).broadcast_to([128, nt, 128]),
                    in1=seg_sb[:, t0 : t0 + nt]
                    .unsqueeze(2)
                    .broadcast_to([128, nt, 128]),
                    op=mybir.AluOpType.is_equal,
                )

            # ---- gather + pooling matmuls (accumulate pooled[seg, H] in PSUM)
            pooled_ps = ppool.tile([128, H], f32, tag="pooled")
            emb_ap = emb_d.ap()
            for ci, (base, t0, nt) in enumerate(calls):
                rows = min(BUCKET, V - base)
                g_sb = gpool.tile([128, CHUNK_TILES, H], f8, tag="g")
                nidx = nt * 128
                nc.gpsimd.dma_gather(
                    out_ap=g_sb[:, :nt, :],
                    in_ap=emb_ap[base : base + rows, :],
                    idxs_ap=idx_sb[:, t0 * 8 : (t0 + nt) * 8],
                    num_idxs=nidx,
                    num_idxs_reg=nidx,
                    elem_size=H,
                    queue_num=ci % GATHER_QUEUES,
                )
                for j in range(nt):
                    t = t0 + j
                    nc.tensor.matmul(
                        pooled_ps[:],
                        lhsT=msk_sb[:, t * 128 : (t + 1) * 128],
                        rhs=g_sb[:, j, :],
                        start=(t == 0),
                        stop=(t == n_tiles - 1),
                    )

            # ---- h = pooled/(len*S): PSUM -> SBUF with the scale fused
            pooled_sb = wpool.tile([128, H], f32, tag="pooled_sb")
            nc.vector.tensor_scalar_mul(pooled_sb[:], pooled_ps[:], leninv_sb[:, :1])

            # ---- transpose to feature-major [H, examples] in PSUM
            hT_ps = tppool.tile([128, 4, 128], f32, tag="hT")
            for c in range(4):
                nc.tensor.transpose(
                    hT_ps[:, c, :], pooled_sb[:, c * 128 : (c + 1) * 128], ident_sb[:]
                )

            # ---- BN batch stats: per-feature sum and sum-of-squares
            hT_sb = wpool.tile([128, 4, 128], f32, tag="hT_sb")
            nc.vector.tensor_copy(hT_sb[:], hT_ps[:])
            nc.vector.tensor_reduce(
                stats_sb[:, 0:4],
                hT_sb[:],
                axis=mybir.AxisListType.X,
                op=mybir.AluOpType.add,
            )
            sq_sb = wpool.tile([128, 4, 128], f32, tag="sq")
            nc.vector.tensor_tensor(
                out=sq_sb[:], in0=hT_sb[:], in1=hT_ps[:], op=mybir.AluOpType.mult
            )
            nc.vector.tensor_reduce(
                stats_sb[:, 4:8],
                sq_sb[:],
                axis=mybir.AxisListType.X,
                op=mybir.AluOpType.add,
            )

            # ---- AllReduce the BN partial sums (4KB, ncfw pre-warmed)
            cc_in = dpool.tile([128, 8], f32, tag="cc_in")
            cc_out = dpool.tile([128, 8], f32, tag="cc_out")
            nc.sync.dma_start(cc_in[:], stats_sb[:])
            nc.gpsimd.collective_compute(
                "AllReduce",
                mybir.AluOpType.add,
                replica_groups=[list(range(NCORES))],
                ins=[cc_in.opt()],
                outs=[cc_out.opt()],
            )
            gstats = wpool.tile([128, 8], f32, tag="gstats")
            nc.sync.dma_start(gstats[:], cc_out[:])

            # ---- BN constants: scale = gamma*rsqrt(var+eps), shift = beta-mu*scale
            t8 = wpool.tile([128, 8], f32, tag="t8")
            nc.vector.tensor_scalar_mul(t8[:], gstats[:], 1.0 / B)
            mu = t8[:, 0:4]
            m2 = t8[:, 4:8]
            var = wpool.tile([128, 4], f32, tag="var")
            nc.vector.tensor_tensor(
                out=var[:], in0=mu, in1=mu, op=mybir.AluOpType.mult
            )
            nc.vector.tensor_tensor(
                out=var[:], in0=m2, in1=var[:], op=mybir.AluOpType.subtract
            )
            eps_sb = wpool.tile([128, 1], f32, tag="eps")
            nc.vector.memset(eps_sb[:], BN_EPS)
            inv = wpool.tile([128, 4], f32, tag="inv")
            nc.scalar.activation(
                out=inv[:],
                in_=var[:],
                func=mybir.ActivationFunctionType.Sqrt,
                bias=eps_sb[:, :1],
                scale=1.0,
            )
            nc.vector.reciprocal(inv[:], inv[:])
            scale_t = wpool.tile([128, 4], f32, tag="scale")
            nc.vector.tensor_tensor(
                out=scale_t[:], in0=inv[:], in1=gamma_sb, op=mybir.AluOpType.mult
            )
            shift = wpool.tile([128, 4], f32, tag="shift")
            nc.vector.tensor_tensor(
                out=shift[:], in0=mu, in1=scale_t[:], op=mybir.AluOpType.mult
            )
            nc.vector.tensor_tensor(
                out=shift[:], in0=beta_sb, in1=shift[:], op=mybir.AluOpType.subtract
            )

            # ---- ReLU(h*scale + shift) straight out of PSUM, final matmul
            hn_sb = wpool.tile([128, 4, 128], f32, tag="hn")
            for m in range(4):
                nc.scalar.activation(
                    out=hn_sb[:, m, :],
                    in_=hT_sb[:, m, :],
                    func=mybir.ActivationFunctionType.Relu,
                    bias=shift[:, m : m + 1],
                    scale=scale_t[:, m : m + 1],
                )
            out_ps = opool.tile([128, 5], f32, tag="out_ps")
            for m in range(4):
                nc.tensor.matmul(
                    out_ps[:],
                    lhsT=hn_sb[:, m, :],
                    rhs=w2_sb[:, m, :],
                    start=(m == 0),
                    stop=(m == 3),
                )
            out_sb = wpool.tile([128, 5], f32, tag="out_sb")
            nc.vector.tensor_tensor(
                out=out_sb[:], in0=out_ps[:], in1=b2_sb, op=mybir.AluOpType.add
            )
            nc.sync.dma_start(out_d[:, :], out_sb[:])

    nc.compile()
    return nc


# ------------------------------------------------------------------- runner
def _prepare(inputs):
    tokens = np.asarray(inputs["tokens"], dtype=np.int32)
    lengths = np.asarray(inputs["lengths"], dtype=np.int32)
    emb = np.asarray(inputs["emb"], dtype=np.float32)
    W1 = np.ascontiguousarray(np.asarray(inputs["W1"], dtype=np.float32))
    gamma = np.asarray(inputs["gamma"], dtype=np.float32)
    beta = np.asarray(inputs["beta"], dtype=np.float32)
    W2 = np.ascontiguousarray(np.asarray(inputs["W2"], dtype=np.float32))
    b2 = np.asarray(inputs["b2"], dtype=np.float32)

    cores, calls, n_tiles, idx16, seg, leninv = _prep(tokens, lengths)

    key = (n_tiles, tuple(calls))
    if key not in _CACHE:
        _CACHE[key] = _build(n_tiles, calls)
    nc = _CACHE[key]

    # host-side fold of W1 into the table; quantize to e3m4
    embW = emb @ W1
    assert np.abs(embW).max() * FP8_SCALE < 15.0, "fp8 e3m4 overflow"
    embWq = np.ascontiguousarray((embW * FP8_SCALE).astype(ml_dtypes.float8_e3m4))

    iota = np.broadcast_to(
        np.arange(128, dtype=np.float32)[None, :], (128, 128)
    ).copy()
    ident = np.eye(128, dtype=np.float32)
    cst = np.zeros((128, 14), dtype=np.float32)
    cst[:, 1:5] = gamma.reshape(4, 128).T
    cst[:, 5:9] = beta.reshape(4, 128).T
    cst[:, 9:14] = b2.reshape(1, 5)
    w2p = np.ascontiguousarray(
        W2.reshape(4, 128, 5).transpose(1, 0, 2).reshape(128, 20)
    )

    in_maps = []
    for c in range(NCORES):
        csta = cst.copy()
        csta[:, 0:1] = leninv[c]
        in_maps.append(
            {
                "embWq": embWq,
                "idx16": np.ascontiguousarray(idx16[c]),
                "seg": np.ascontiguousarray(seg[c]),
                "iota": iota,
                "cst": csta,
                "W2p": w2p,
                "ident": ident,
            }
        )
    return nc, in_maps, cores


def _run(inputs, trace=False):
    nc, in_maps, cores = _prepare(inputs)

    from concourse.bass_utils import run_bass_kernel_spmd

    res = run_bass_kernel_spmd(
        nc, in_maps, core_ids=list(range(NCORES)), trace=trace
    )

    out = np.zeros((B, 5), dtype=np.float32)
    for c in range(NCORES):
        out[np.asarray(cores[c])] = res.results[c]["out"]
    return out, res


def kernel(**inputs) -> np.ndarray:
    out, _ = _run(inputs, trace=False)
    return out
